# revision 1
# baseline (speedup 1.0000x reference)
"""Trainium2 Bass kernel for ContextualAttentionBlock.

Sharding: 8 cores, core c -> (batch b = c//2, query-half qh = c%2).
Each core computes, for its batch's 1024-token attention window:
  K/V projections for all 1024 tokens, Q for its 512 queries, RoPE,
  attention, out-proj, residual+RMSNorm1 -> h1 (512 tokens),
then SwiGLU FFN + residual + RMSNorm2 for 2048 tokens
  (512 attention-part tokens + 1536 "rest" tokens that skip attention).
All activations are kept feature-major ([feature, token]) so every matmul
contracts over the partition dim.  Matmuls run as float32r (TF32).
No collectives; the host shards inputs and reassembles the output.
"""

import numpy as np

import concourse.bass as bass
import concourse.tile as tile
from concourse import bacc, mybir
from concourse.bass_utils import run_bass_kernel_spmd

F32 = mybir.dt.float32
F32R = mybir.dt.float32r
AF = mybir.ActivationFunctionType
OP = mybir.AluOpType

WIDTH = 1024
NT = 8              # width tiles of 128
HEADS = 16
HDIM = 64
LWIN = 1024         # attention window
LQ = 512            # queries per core
HID = 4096
NH = 32             # hidden tiles of 128
REST = 1536         # rest tokens per core
CH = 512            # ffn token chunk
NCH = 4             # ffn chunks: LQ + REST = 2048 = 4*512
TOUT = LQ + REST
EPS = 1e-6
ROPE_BASE = 10000.0
N_CORES = 8


def _r(ap):
    return ap.bitcast(mybir.dt.float32r)


def _emit(tc, A, out_ap):
    nc = tc.nc
    mm = nc.tensor.matmul

    xw_r = A["xw_t"].rearrange("(a p) t -> a p t", p=128)
    xq_r = A["xq_t"].rearrange("(a p) t -> a p t", p=128)
    xr_r = A["xr_t"].rearrange("(a p) t -> a p t", p=128)
    wq_r = A["wq_t"].rearrange("(a p) m -> p a m", p=128)
    wk_r = A["wk_t"].rearrange("(a p) m -> p a m", p=128)
    wv_r = A["wv_t"].rearrange("(a p) m -> p a m", p=128)
    wo_r = A["wo_t"].rearrange("(a p) m -> p a m", p=128)
    wg_r = A["wg_t"].rearrange("(a p) m -> p a m", p=128)
    wu_r = A["wu_t"].rearrange("(a p) m -> p a m", p=128)
    wd_r = A["wd_t"].rearrange("(a p) m -> p a m", p=128)
    out_r = out_ap.rearrange("(a p) t -> a p t", p=128)

    with tc.tile_pool(name="pc", bufs=1) as pc:
        cq = pc.tile([128, LQ], F32R, name="cq")
        nc.sync.dma_start(cq, A["cos_q"])
        sq = pc.tile([128, LQ], F32R, name="sq")
        nc.sync.dma_start(sq, A["sin_q"])
        ck = pc.tile([128, LWIN], F32R, name="ck")
        nc.sync.dma_start(ck, A["cos_k"])
        sk = pc.tile([128, LWIN], F32R, name="sk")
        nc.sync.dma_start(sk, A["sin_k"])
        g1 = pc.tile([128, NT], F32R, name="g1")
        nc.sync.dma_start(g1, A["g1"])
        g2 = pc.tile([128, NT], F32R, name="g2")
        nc.sync.dma_start(g2, A["g2"])
        onesF = pc.tile([128, 128], F32, name="onesF")
        nc.vector.memset(onesF, 1.0)
        onesK = pc.tile([128, 1], F32R, name="onesK")
        nc.vector.tensor_copy(onesK, onesF[:, 0:1])
        ones1 = pc.tile([1, 128], F32R, name="ones1")
        nc.vector.tensor_copy(ones1, onesF[0:1, :])
        eps1 = pc.tile([1, 1], F32, name="eps1")
        nc.vector.memset(eps1, EPS)
        ones64 = pc.tile([1, 64], F32R, name="ones64")
        nc.vector.tensor_copy(ones64, onesF[0:1, 0:64])
        h1 = [pc.tile([128, LQ], F32R, name=f"h1_{m}", tag=f"h1_{m}") for m in range(NT)]
        xq = [pc.tile([128, LQ], F32R, name=f"xq_{k}", tag=f"xq_{k}") for k in range(NT)]
        for k in range(NT):
            nc.sync.dma_start(xq[k], xq_r[k])

        # ---------------- Stage A: attention ----------------
        with tc.tile_pool(name="pb", bufs=1) as pb:
            ao = [pb.tile([128, LQ], F32R, name=f"ao_{i}", tag=f"ao{i}") for i in range(NT)]
            s_sb = [pb.tile([128, LQ], F32R, name=f"s_{m}", tag=f"s{m}") for m in range(NT)]
            with tc.tile_pool(name="pa", bufs=1) as pa, \
                 tc.tile_pool(name="wa", bufs=1) as wa, \
                 tc.tile_pool(name="psa", bufs=1, space="PSUM") as psa:
                xw = [pa.tile([128, LWIN], F32R, name=f"xw_{k}", tag=f"xw{k}") for k in range(NT)]
                for k in range(NT):
                    nc.sync.dma_start(xw[k], xw_r[k])

                def rope_from_psum(ps, dest, cos, sin, scratch, T):
                    # per 64-row head block: rows b..b+32 = even dims E,
                    # rows b+32..b+64 = odd dims O (head-contiguous perm).
                    # dest[b:b+32]    = E*cos - O*sin
                    # dest[b+32:b+64] = O*cos + E*sin
                    # cos/sin rows repeat every 32 (freq = row % 32), so the
                    # SBUF operand can always share the output's base partition.
                    for b in (0, 64):
                        e_ps, o_ps = ps[b:b + 32, :], ps[b + 32:b + 64, :]
                        nc.vector.tensor_mul(scratch[b + 32:b + 64, :], e_ps,
                                             sin[b + 32:b + 64, :])  # E*sin
                        nc.vector.tensor_mul(scratch[b:b + 32, :], o_ps,
                                             sin[b:b + 32, :])       # O*sin
                        nc.vector.tensor_mul(dest[b:b + 32, :], e_ps, cos[b:b + 32, :])
                        nc.vector.tensor_sub(dest[b:b + 32, :], dest[b:b + 32, :],
                                             scratch[b:b + 32, :])
                        nc.vector.tensor_mul(dest[b + 32:b + 64, :], o_ps,
                                             cos[b + 32:b + 64, :])
                        nc.vector.tensor_add(dest[b + 32:b + 64, :],
                                             dest[b + 32:b + 64, :],
                                             scratch[b + 32:b + 64, :])

                for g in range(4):
                    # --- Q projection + RoPE (tiles 2g, 2g+1; heads 4g..4g+3) ---
                    q2 = []
                    for side, m in ((0, 2 * g), (1, 2 * g + 1)):
                        wb = wa.tile([128, NT, 128], F32R, name=f"wqb_{g}_{side}", tag="wqk", bufs=2)
                        nc.sync.dma_start(wb, wq_r[:, :, m * 128:(m + 1) * 128])
                        ps = psa.tile([128, LQ], F32, name=f"qps_{g}_{side}", tag="proj", bufs=2)
                        for k in range(NT):
                            mm(ps, _r(wb[:, k, :]), _r(xq[k]), start=(k == 0), stop=(k == NT - 1))
                        qt = pa.tile([128, LQ], F32R, name=f"q_{g}_{side}",
                                     tag=("q0" if side == 0 else "q1"), bufs=2)
                        qs = pa.tile([128, LQ], F32R, name=f"qs_{g}_{side}", tag="rs", bufs=2)
                        rope_from_psum(ps, qt, cq, sq, qs, LQ)
                        q2.append(qt)

                    # --- K projection + RoPE ---
                    k2 = []
                    for side, m in ((0, 2 * g), (1, 2 * g + 1)):
                        wb = wa.tile([128, NT, 128], F32R, name=f"wkb_{g}_{side}", tag="wqk", bufs=2)
                        nc.sync.dma_start(wb, wk_r[:, :, m * 128:(m + 1) * 128])
                        kt_sb = pa.tile([128, LWIN], F32R, name=f"k_{g}_{side}",
                                        tag=("k0" if side == 0 else "k1"), bufs=2)
                        for ch in range(2):
                            ps = psa.tile([128, 512], F32, name=f"kps_{g}_{side}_{ch}", tag="proj", bufs=2)
                            for k in range(NT):
                                mm(ps, _r(wb[:, k, :]), _r(xw[k][:, ch * 512:(ch + 1) * 512]),
                                   start=(k == 0), stop=(k == NT - 1))
                            ks = pa.tile([128, 512], F32R, name=f"ks_{g}_{side}_{ch}", tag="rs", bufs=2)
                            rope_from_psum(ps, kt_sb[:, ch * 512:(ch + 1) * 512],
                                           ck[:, ch * 512:(ch + 1) * 512],
                                           sk[:, ch * 512:(ch + 1) * 512], ks, 512)
                        k2.append(kt_sb)

                    # --- V projection (token-major, 65-col per head with ones col) ---
                    wvb = wa.tile([128, NT, 256], F32R, name=f"wvb_{g}", tag="wv", bufs=1)
                    nc.sync.dma_start(wvb, wv_r[:, :, g * 256:(g + 1) * 256])
                    vg = [pa.tile([128, 4 * 65], F32R, name=f"v_{g}_{kt}", tag=f"v{kt}", bufs=2)
                          for kt in range(NT)]
                    for kt in range(NT):
                        psv = psa.tile([128, 256], F32, name=f"vps_{g}_{kt}", tag="proj", bufs=2)
                        for k in range(NT):
                            mm(psv, _r(xw[k][:, kt * 128:(kt + 1) * 128]), _r(wvb[:, k, :]),
                               start=(k == 0), stop=(k == NT - 1))
                        v3 = vg[kt].rearrange("p (h c) -> p h c", c=65)
                        nc.vector.tensor_copy(v3[:, :, 64:65],
                                              onesF[:, 0:1].unsqueeze(1).broadcast_to([128, 4, 1]))
                        nc.vector.tensor_copy(v3[:, :, 0:64], psv.rearrange("p (h c) -> p h c", c=64))

                    # --- attention per head pair ---
                    for p2 in range(2):
                        vac = [psa.tile([65, 512], F32, name=f"vac_{g}_{p2}_{jj}", tag="vac", bufs=4)
                               for jj in range(2)]
                        for kt in range(NT):
                            for jj in range(2):
                                j = 2 * p2 + jj
                                sc = psa.tile([128, 512], F32, name=f"sc_{g}_{p2}_{kt}_{jj}",
                                              tag="sc", bufs=2)
                                mm(sc, _r(k2[p2][64 * jj:64 * (jj + 1), kt * 128:(kt + 1) * 128]),
                                   _r(q2[p2][64 * jj:64 * (jj + 1), :]),
                                   start=True, stop=True, tile_position=(64 * jj, 0))
                                at = pa.tile([128, 512], F32R, name=f"at_{g}_{p2}_{kt}_{jj}",
                                             tag="at", bufs=3)
                                nc.scalar.activation(at, sc, AF.Exp, scale=0.125)
                                mm(vac[jj], _r(vg[kt][:, j * 65:(j + 1) * 65]), _r(at),
                                   start=(kt == 0), stop=(kt == NT - 1))
                        # normalize pair -> attention out tile i (heads 2i, 2i+1)
                        rr = pa.tile([1, 1024], F32R, name=f"rr_{g}_{p2}", tag="rr", bufs=2)
                        nc.scalar.copy(rr[0:1, 0:512], vac[0][64:65, :])
                        nc.scalar.copy(rr[0:1, 512:1024], vac[1][64:65, :])
                        with nc.allow_low_precision(reason="tf32 softmax denom"):
                            nc.vector.reciprocal(rr, rr)
                        bc0 = psa.tile([64, 512], F32, name=f"bca_{g}_{p2}_0", tag="vac", bufs=4)
                        mm(bc0, _r(ones64), _r(rr[0:1, 0:512]))
                        bc1 = psa.tile([64, 512], F32, name=f"bca_{g}_{p2}_1", tag="vac", bufs=4)
                        mm(bc1, _r(ones64), _r(rr[0:1, 512:1024]))
                        bcs = pa.tile([128, 512], F32R, name=f"bcs_{g}_{p2}", tag="bcs", bufs=1)
                        nc.scalar.copy(bcs[0:64, :], bc0)
                        nc.scalar.copy(bcs[64:128, :], bc1)
                        i = 2 * g + p2
                        nc.vector.tensor_mul(ao[i][0:64, :], vac[0][0:64, :], bcs[0:64, :])
                        nc.vector.tensor_mul(ao[i][64:128, :], vac[1][0:64, :], bcs[64:128, :])

                # ---------------- Stage B: out-proj + RMSNorm1 -> h1 ----------------
                ssp = psa.tile([1, 512], F32, name="ssp", tag="sc", bufs=2)
                for m in range(NT):
                    wb = wa.tile([128, NT, 128], F32R, name=f"wob_{m}", tag="wqk", bufs=2)
                    nc.sync.dma_start(wb, wo_r[:, :, m * 128:(m + 1) * 128])
                    yp = psa.tile([128, LQ], F32, name=f"yps_{m}", tag="proj", bufs=2)
                    for k in range(NT):
                        mm(yp, _r(wb[:, k, :]), _r(ao[k]), start=(k == 0), stop=(k == NT - 1))
                    nc.vector.tensor_add(s_sb[m], xq[m], yp)
                    sqt = pa.tile([128, LQ], F32R, name=f"sq1_{m}", tag="sq", bufs=2)
                    nc.vector.tensor_mul(sqt, s_sb[m], s_sb[m])
                    mm(ssp, _r(onesK), _r(sqt), start=(m == 0), stop=(m == NT - 1))
                row = pa.tile([1, 512], F32R, name="row1", tag="row", bufs=2)
                nc.scalar.activation(row, ssp, AF.Sqrt, scale=1.0 / WIDTH, bias=eps1)
                with nc.allow_low_precision(reason="tf32 rstd"):
                    nc.vector.reciprocal(row, row)
                bcn = psa.tile([128, 512], F32, name="bcn", tag="vac", bufs=4)
                mm(bcn, _r(ones1), _r(row))
                for m in range(NT):
                    nc.vector.scalar_tensor_tensor(h1[m], s_sb[m], g1[:, m:m + 1], bcn,
                                                   op0=OP.mult, op1=OP.mult)

        # ---------------- Stage C: SwiGLU FFN + RMSNorm2 ----------------
        with tc.tile_pool(name="pcn", bufs=1) as pcn, \
             tc.tile_pool(name="wc", bufs=1) as wc, \
             tc.tile_pool(name="psc", bufs=1, space="PSUM") as psc:
            F_t = [pcn.tile([128, CH], F32R, name=f"F_{h}", tag=f"F{h}") for h in range(NH)]
            for c in range(NCH):
                if c == 0:
                    tin = h1
                else:
                    tin = [pcn.tile([128, CH], F32R, name=f"tin_{c}_{k}", tag=f"tin{k}", bufs=2)
                           for k in range(NT)]
                    for k in range(NT):
                        nc.sync.dma_start(tin[k], xr_r[k][:, (c - 1) * CH:c * CH])
                for hm in range(NH):
                    wgb = wc.tile([128, NT, 128], F32R, name=f"wgb_{c}_{hm}", tag="wgu", bufs=4)
                    nc.sync.dma_start(wgb, wg_r[:, :, hm * 128:(hm + 1) * 128])
                    wub = wc.tile([128, NT, 128], F32R, name=f"wub_{c}_{hm}", tag="wgu", bufs=4)
                    nc.sync.dma_start(wub, wu_r[:, :, hm * 128:(hm + 1) * 128])
                    gp = psc.tile([128, CH], F32, name=f"gp_{c}_{hm}", tag="gu", bufs=3)
                    up = psc.tile([128, CH], F32, name=f"up_{c}_{hm}", tag="gu", bufs=3)
                    for k in range(NT):
                        mm(gp, _r(wgb[:, k, :]), _r(tin[k]), start=(k == 0), stop=(k == NT - 1))
                    for k in range(NT):
                        mm(up, _r(wub[:, k, :]), _r(tin[k]), start=(k == 0), stop=(k == NT - 1))
                    sg = pcn.tile([128, CH], F32R, name=f"sg_{c}_{hm}", tag="sg", bufs=2)
                    nc.scalar.activation(sg, gp, AF.Silu)
                    nc.vector.tensor_mul(F_t[hm], sg, up)
                s2l = [pcn.tile([128, CH], F32R, name=f"s2_{c}_{m}", tag=f"s2_{m}") for m in range(NT)]
                ssp2 = psc.tile([1, CH], F32, name=f"ssp2_{c}", tag="st", bufs=1)
                for m in range(NT):
                    dp = psc.tile([128, CH], F32, name=f"dp_{c}_{m}", tag="d", bufs=2)
                    for hh in range(2):
                        wdb = wc.tile([128, 16, 128], F32R, name=f"wdb_{c}_{m}_{hh}", tag="wd", bufs=2)
                        nc.sync.dma_start(wdb, wd_r[:, hh * 16:(hh + 1) * 16, m * 128:(m + 1) * 128])
                        for h2 in range(16):
                            hm = hh * 16 + h2
                            mm(dp, _r(wdb[:, h2, :]), _r(F_t[hm]), start=(hm == 0), stop=(hm == NH - 1))
                    nc.vector.tensor_add(s2l[m], tin[m], dp)
                    sqt = pcn.tile([128, CH], F32R, name=f"sq2_{c}_{m}", tag="sg", bufs=2)
                    nc.vector.tensor_mul(sqt, s2l[m], s2l[m])
                    mm(ssp2, _r(onesK), _r(sqt), start=(m == 0), stop=(m == NT - 1))
                row2 = pcn.tile([1, CH], F32R, name=f"row2_{c}", tag="row2", bufs=2)
                nc.scalar.activation(row2, ssp2, AF.Sqrt, scale=1.0 / WIDTH, bias=eps1)
                with nc.allow_low_precision(reason="tf32 rstd"):
                    nc.vector.reciprocal(row2, row2)
                bc2 = psc.tile([128, CH], F32, name=f"bc2_{c}", tag="d", bufs=2)
                mm(bc2, _r(ones1), _r(row2))
                for m in range(NT):
                    ot = pcn.tile([128, CH], F32R, name=f"ot_{c}_{m}", tag="ot", bufs=2)
                    nc.vector.scalar_tensor_tensor(ot, s2l[m], g2[:, m:m + 1], bc2,
                                                   op0=OP.mult, op1=OP.mult)
                    nc.sync.dma_start(out_r[m][:, c * CH:(c + 1) * CH], ot)


_INPUT_SPECS = [
    ("xw_t", [WIDTH, LWIN]),
    ("xq_t", [WIDTH, LQ]),
    ("xr_t", [WIDTH, REST]),
    ("wq_t", [WIDTH, WIDTH]),
    ("wk_t", [WIDTH, WIDTH]),
    ("wv_t", [WIDTH, WIDTH]),
    ("wo_t", [WIDTH, WIDTH]),
    ("wg_t", [WIDTH, HID]),
    ("wu_t", [WIDTH, HID]),
    ("wd_t", [HID, WIDTH]),
    ("cos_q", [128, LQ]),
    ("sin_q", [128, LQ]),
    ("cos_k", [128, LWIN]),
    ("sin_k", [128, LWIN]),
    ("g1", [128, NT]),
    ("g2", [128, NT]),
]


def build_program(reps=1):
    nc = bacc.Bacc("TRN2", target_bir_lowering=False, debug=False, num_devices=N_CORES)
    A = {name: nc.dram_tensor(name, shape, F32R, kind="ExternalInput").ap()
         for name, shape in _INPUT_SPECS}
    out_ap = nc.dram_tensor("out_t", [WIDTH, TOUT], F32R, kind="ExternalOutput").ap()
    with tile.TileContext(nc) as tc:
        for _ in range(reps):
            _emit(tc, A, out_ap)
    nc.compile()
    return nc


def make_in_maps(x, w_qkv, w_out, g_norm1, g_norm2, w_gate, w_up, w_down):
    f32 = np.float32
    x = np.asarray(x, f32)
    w_qkv = np.asarray(w_qkv, f32)
    # head-contiguous de-interleave: within head h, even dims first then odd:
    # new row h*64+j -> old h*64+2j ; new row h*64+32+j -> old h*64+2j+1
    perm = np.empty(WIDTH, np.int64)
    for h in range(HEADS):
        j = np.arange(32)
        perm[h * 64 + j] = h * 64 + 2 * j
        perm[h * 64 + 32 + j] = h * 64 + 2 * j + 1
    wq = w_qkv[0:WIDTH][perm]
    wk = w_qkv[WIDTH:2 * WIDTH][perm]
    wv = w_qkv[2 * WIDTH:3 * WIDTH]

    inv_freq = (1.0 / (ROPE_BASE ** (np.arange(0, HDIM, 2, dtype=np.float64) / HDIM)))

    def tab(pos):
        fr = np.outer(inv_freq, pos.astype(np.float64))  # [32, T]
        return (np.tile(np.cos(fr), (4, 1)).astype(f32),
                np.tile(np.sin(fr), (4, 1)).astype(f32))

    cos_k, sin_k = tab(np.arange(LWIN))
    common = {
        "wq_t": np.ascontiguousarray(wq.T),
        "wk_t": np.ascontiguousarray(wk.T),
        "wv_t": np.ascontiguousarray(wv.T),
        "wo_t": np.ascontiguousarray(np.asarray(w_out, f32).T),
        "wg_t": np.ascontiguousarray(np.asarray(w_gate, f32).T),
        "wu_t": np.ascontiguousarray(np.asarray(w_up, f32).T),
        "wd_t": np.ascontiguousarray(np.asarray(w_down, f32).T),
        "cos_k": cos_k,
        "sin_k": sin_k,
        "g1": np.ascontiguousarray(np.asarray(g_norm1, f32).reshape(NT, 128).T),
        "g2": np.ascontiguousarray(np.asarray(g_norm2, f32).reshape(NT, 128).T),
    }
    in_maps = []
    for c in range(N_CORES):
        b, qh = c // 2, c % 2
        cos_q, sin_q = tab(np.arange(qh * LQ, (qh + 1) * LQ))
        m = dict(common)
        m["xw_t"] = np.ascontiguousarray(x[b, :LWIN].T)
        m["xq_t"] = np.ascontiguousarray(x[b, qh * LQ:(qh + 1) * LQ].T)
        m["xr_t"] = np.ascontiguousarray(x[b, LWIN + qh * REST:LWIN + (qh + 1) * REST].T)
        m["cos_q"] = cos_q
        m["sin_q"] = sin_q
        in_maps.append(m)
    return in_maps


def assemble_output(results):
    out = np.empty((4, 4096, WIDTH), np.float32)
    for c in range(N_CORES):
        b, qh = c // 2, c % 2
        o = results[c]["out_t"]
        out[b, qh * LQ:(qh + 1) * LQ] = o[:, :LQ].T
        out[b, LWIN + qh * REST:LWIN + (qh + 1) * REST] = o[:, LQ:].T
    return out


_CACHE = {}


def kernel(x, w_qkv, w_out, g_norm1, g_norm2, w_gate, w_up, w_down):
    if "nc" not in _CACHE:
        _CACHE["nc"] = build_program()
    nc = _CACHE["nc"]
    in_maps = make_in_maps(x, w_qkv, w_out, g_norm1, g_norm2, w_gate, w_up, w_down)
    res = run_bass_kernel_spmd(nc, in_maps, list(range(N_CORES))).results
    return assemble_output(res)



# revision 2
# speedup vs baseline: 1.2176x; 1.2176x over previous
"""Trainium2 Bass kernel for ContextualAttentionBlock.

Sharding: 8 cores, core c -> (batch b = c//2, query-half qh = c%2).
Each core computes, for its batch's 1024-token attention window:
  K/V projections for all 1024 tokens, Q for its 512 queries, RoPE,
  attention, out-proj, residual+RMSNorm1 -> h1 (512 tokens),
then SwiGLU FFN + residual + RMSNorm2 for 2048 tokens
  (512 attention-part tokens + 1536 "rest" tokens that skip attention).

All activations are feature-major ([feature, token]) so every matmul
contracts over the partition dim.  Matmul operands are bf16 (weights
cast on host; activations cast on device as they leave PSUM), which
halves HBM traffic and weight-load time; accumulation stays fp32.
The FFN streams gate/up/down weights once per 1024-token block
(hidden-major loop) instead of once per 512-token chunk, and the
pure-"rest" block is emitted first so the scheduler can fill
attention-stage PE gaps with FFN matmuls.
No collectives; the host shards inputs and reassembles the output.
"""

import numpy as np
import ml_dtypes

import concourse.bass as bass
import concourse.tile as tile
from concourse import bacc, mybir
from concourse.bass_utils import run_bass_kernel_spmd

F32 = mybir.dt.float32
F32R = mybir.dt.float32r
BF16 = mybir.dt.bfloat16
AF = mybir.ActivationFunctionType
OP = mybir.AluOpType

WIDTH = 1024
NT = 8              # width tiles of 128
HEADS = 16
HDIM = 64
LWIN = 1024         # attention window
LQ = 512            # queries per core
HID = 4096
NH = 32             # hidden tiles of 128
REST = 1536         # rest tokens per core
TBLK = 1024         # ffn token block
NJ = 2              # 512-token slices per block
TOUT = LQ + REST
EPS = 1e-6
ROPE_BASE = 10000.0
N_CORES = 8


def _r(ap):
    return ap.bitcast(mybir.dt.float32r)


def _emit(tc, A, out_ap):
    nc = tc.nc
    mm = nc.tensor.matmul

    xw_r = A["xw_t"].rearrange("(a p) t -> a p t", p=128)
    xq_r = A["xq_t"].rearrange("(a p) t -> a p t", p=128)
    xr_r = A["xr_t"].rearrange("(a p) t -> a p t", p=128)
    wq_r = A["wq_t"].rearrange("(a p) m -> p a m", p=128)
    wk_r = A["wk_t"].rearrange("(a p) m -> p a m", p=128)
    wv_r = A["wv_t"].rearrange("(a p) m -> p a m", p=128)
    wo_r = A["wo_t"].rearrange("(a p) m -> p a m", p=128)
    wg_r = A["wg_t"].rearrange("(a p) m -> p a m", p=128)
    wu_r = A["wu_t"].rearrange("(a p) m -> p a m", p=128)
    wd_r = A["wd_t"].rearrange("(a p) m -> p a m", p=128)
    out_r = out_ap.rearrange("(a p) t -> a p t", p=128)

    with tc.tile_pool(name="pc", bufs=1) as pc:
        g1 = pc.tile([128, NT], F32R, name="g1")
        nc.sync.dma_start(g1, A["g1"])
        g2 = pc.tile([128, NT], F32R, name="g2")
        nc.sync.dma_start(g2, A["g2"])
        onesF = pc.tile([128, 128], F32, name="onesF")
        nc.vector.memset(onesF, 1.0)
        onesK = pc.tile([128, 1], F32R, name="onesK")
        nc.vector.tensor_copy(onesK, onesF[:, 0:1])
        ones1 = pc.tile([1, 128], F32R, name="ones1")
        nc.vector.tensor_copy(ones1, onesF[0:1, :])
        eps1 = pc.tile([1, 1], F32, name="eps1")
        nc.vector.memset(eps1, EPS)
        ones64 = pc.tile([1, 64], F32R, name="ones64")
        nc.vector.tensor_copy(ones64, onesF[0:1, 0:64])
        # FFN input, feature-major: cols [0:LQ] = h1 (written by stage B),
        # cols [LQ:] = raw rest tokens (DMA'd immediately).
        tin = [pc.tile([128, TOUT], BF16, name=f"tin_{m}", tag=f"tin_{m}")
               for m in range(NT)]
        for m in range(NT):
            nc.sync.dma_start(tin[m][:, LQ:], xr_r[m])

        # ---------------- Stage A: attention ----------------
        with tc.tile_pool(name="pb", bufs=1) as pb:
            cq = pb.tile([128, LQ], F32, name="cq")
            nc.sync.dma_start(cq, A["cos_q"])
            sq = pb.tile([128, LQ], F32, name="sq")
            nc.sync.dma_start(sq, A["sin_q"])
            ck = pb.tile([128, LWIN], F32, name="ck")
            nc.sync.dma_start(ck, A["cos_k"])
            sk = pb.tile([128, LWIN], F32, name="sk")
            nc.sync.dma_start(sk, A["sin_k"])
            xq = [pb.tile([128, LQ], BF16, name=f"xq_{k}", tag=f"xq_{k}") for k in range(NT)]
            for k in range(NT):
                nc.sync.dma_start(xq[k], xq_r[k])
            ao = [pb.tile([128, LQ], BF16, name=f"ao_{i}", tag=f"ao{i}") for i in range(NT)]
            s_sb = [pb.tile([128, LQ], F32R, name=f"s_{m}", tag=f"s{m}") for m in range(NT)]
            with tc.tile_pool(name="pa", bufs=1) as pa, \
                 tc.tile_pool(name="wa", bufs=1) as wa, \
                 tc.tile_pool(name="psa", bufs=1, space="PSUM") as psa:
                xw = [pa.tile([128, LWIN], BF16, name=f"xw_{k}", tag=f"xw{k}") for k in range(NT)]
                for k in range(NT):
                    nc.sync.dma_start(xw[k], xw_r[k])

                def rope_from_psum(ps, dest, cos, sin, scratch, T):
                    # per 64-row head block: rows b..b+32 = even dims E,
                    # rows b+32..b+64 = odd dims O (head-contiguous perm).
                    # dest[b:b+32]    = E*cos - O*sin
                    # dest[b+32:b+64] = O*cos + E*sin
                    # cos/sin rows repeat every 32 (freq = row % 32), so the
                    # SBUF operand can always share the output's base partition.
                    for b in (0, 64):
                        e_ps, o_ps = ps[b:b + 32, :], ps[b + 32:b + 64, :]
                        nc.vector.tensor_mul(scratch[b + 32:b + 64, :], e_ps,
                                             sin[b + 32:b + 64, :])  # E*sin
                        nc.vector.tensor_mul(scratch[b:b + 32, :], o_ps,
                                             sin[b:b + 32, :])       # O*sin
                        nc.vector.tensor_mul(dest[b:b + 32, :], e_ps, cos[b:b + 32, :])
                        nc.vector.tensor_sub(dest[b:b + 32, :], dest[b:b + 32, :],
                                             scratch[b:b + 32, :])
                        nc.vector.tensor_mul(dest[b + 32:b + 64, :], o_ps,
                                             cos[b + 32:b + 64, :])
                        nc.vector.tensor_add(dest[b + 32:b + 64, :],
                                             dest[b + 32:b + 64, :],
                                             scratch[b + 32:b + 64, :])

                for g in range(4):
                    # --- Q projection + RoPE (tiles 2g, 2g+1; heads 4g..4g+3) ---
                    q2 = []
                    for side, m in ((0, 2 * g), (1, 2 * g + 1)):
                        wb = wa.tile([128, NT, 128], BF16, name=f"wqb_{g}_{side}", tag="wqk", bufs=2)
                        nc.sync.dma_start(wb, wq_r[:, :, m * 128:(m + 1) * 128])
                        ps = psa.tile([128, LQ], F32, name=f"qps_{g}_{side}", tag="proj", bufs=2)
                        for k in range(NT):
                            mm(ps, wb[:, k, :], xq[k], start=(k == 0), stop=(k == NT - 1))
                        qt = pa.tile([128, LQ], BF16, name=f"q_{g}_{side}",
                                     tag=("q0" if side == 0 else "q1"), bufs=2)
                        qs = pa.tile([128, LQ], F32R, name=f"qs_{g}_{side}", tag="rs", bufs=2)
                        rope_from_psum(ps, qt, cq, sq, qs, LQ)
                        q2.append(qt)

                    # --- K projection + RoPE ---
                    k2 = []
                    for side, m in ((0, 2 * g), (1, 2 * g + 1)):
                        wb = wa.tile([128, NT, 128], BF16, name=f"wkb_{g}_{side}", tag="wqk", bufs=2)
                        nc.sync.dma_start(wb, wk_r[:, :, m * 128:(m + 1) * 128])
                        kt_sb = pa.tile([128, LWIN], BF16, name=f"k_{g}_{side}",
                                        tag=("k0" if side == 0 else "k1"), bufs=2)
                        for ch in range(2):
                            ps = psa.tile([128, 512], F32, name=f"kps_{g}_{side}_{ch}", tag="proj", bufs=2)
                            for k in range(NT):
                                mm(ps, wb[:, k, :], xw[k][:, ch * 512:(ch + 1) * 512],
                                   start=(k == 0), stop=(k == NT - 1))
                            ks = pa.tile([128, 512], F32R, name=f"ks_{g}_{side}_{ch}", tag="rs", bufs=2)
                            rope_from_psum(ps, kt_sb[:, ch * 512:(ch + 1) * 512],
                                           ck[:, ch * 512:(ch + 1) * 512],
                                           sk[:, ch * 512:(ch + 1) * 512], ks, 512)
                        k2.append(kt_sb)

                    # --- V projection (token-major, 65-col per head with ones col) ---
                    wvb = wa.tile([128, NT, 256], BF16, name=f"wvb_{g}", tag="wv", bufs=1)
                    nc.sync.dma_start(wvb, wv_r[:, :, g * 256:(g + 1) * 256])
                    vg = [pa.tile([128, 4 * 65], BF16, name=f"v_{g}_{kt}", tag=f"v{kt}", bufs=2)
                          for kt in range(NT)]
                    for kt in range(NT):
                        psv = psa.tile([128, 256], F32, name=f"vps_{g}_{kt}", tag="proj", bufs=2)
                        for k in range(NT):
                            mm(psv, xw[k][:, kt * 128:(kt + 1) * 128], wvb[:, k, :],
                               start=(k == 0), stop=(k == NT - 1))
                        v3 = vg[kt].rearrange("p (h c) -> p h c", c=65)
                        nc.vector.tensor_copy(v3[:, :, 64:65],
                                              onesF[:, 0:1].unsqueeze(1).broadcast_to([128, 4, 1]))
                        nc.vector.tensor_copy(v3[:, :, 0:64], psv.rearrange("p (h c) -> p h c", c=64))

                    # --- attention per head pair ---
                    for p2 in range(2):
                        vac = [psa.tile([65, 512], F32, name=f"vac_{g}_{p2}_{jj}", tag="vac", bufs=4)
                               for jj in range(2)]
                        for kt in range(NT):
                            for jj in range(2):
                                j = 2 * p2 + jj
                                sc = psa.tile([128, 512], F32, name=f"sc_{g}_{p2}_{kt}_{jj}",
                                              tag="sc", bufs=2)
                                mm(sc, k2[p2][64 * jj:64 * (jj + 1), kt * 128:(kt + 1) * 128],
                                   q2[p2][64 * jj:64 * (jj + 1), :],
                                   start=True, stop=True, tile_position=(64 * jj, 0))
                                at = pa.tile([128, 512], BF16, name=f"at_{g}_{p2}_{kt}_{jj}",
                                             tag="at", bufs=4)
                                nc.scalar.activation(at, sc, AF.Exp, scale=0.125)
                                mm(vac[jj], vg[kt][:, j * 65:(j + 1) * 65], at,
                                   start=(kt == 0), stop=(kt == NT - 1))
                        # normalize pair -> attention out tile i (heads 2i, 2i+1)
                        rr = pa.tile([1, 1024], F32R, name=f"rr_{g}_{p2}", tag="rr", bufs=2)
                        nc.scalar.copy(rr[0:1, 0:512], vac[0][64:65, :])
                        nc.scalar.copy(rr[0:1, 512:1024], vac[1][64:65, :])
                        with nc.allow_low_precision(reason="tf32 softmax denom"):
                            nc.vector.reciprocal(rr, rr)
                        bc0 = psa.tile([64, 512], F32, name=f"bca_{g}_{p2}_0", tag="vac", bufs=4)
                        mm(bc0, _r(ones64), _r(rr[0:1, 0:512]))
                        bc1 = psa.tile([64, 512], F32, name=f"bca_{g}_{p2}_1", tag="vac", bufs=4)
                        mm(bc1, _r(ones64), _r(rr[0:1, 512:1024]))
                        bcs = pa.tile([128, 512], F32R, name=f"bcs_{g}_{p2}", tag="bcs", bufs=1)
                        nc.scalar.copy(bcs[0:64, :], bc0)
                        nc.scalar.copy(bcs[64:128, :], bc1)
                        i = 2 * g + p2
                        nc.vector.tensor_mul(ao[i][0:64, :], vac[0][0:64, :], bcs[0:64, :])
                        nc.vector.tensor_mul(ao[i][64:128, :], vac[1][0:64, :], bcs[64:128, :])

                # ---------------- Stage B: out-proj + RMSNorm1 -> h1 ----------------
                ssp = psa.tile([1, 512], F32, name="ssp", tag="sc", bufs=2)
                for m in range(NT):
                    wb = wa.tile([128, NT, 128], BF16, name=f"wob_{m}", tag="wqk", bufs=2)
                    nc.sync.dma_start(wb, wo_r[:, :, m * 128:(m + 1) * 128])
                    yp = psa.tile([128, LQ], F32, name=f"yps_{m}", tag="proj", bufs=2)
                    for k in range(NT):
                        mm(yp, wb[:, k, :], ao[k], start=(k == 0), stop=(k == NT - 1))
                    nc.vector.tensor_add(s_sb[m], xq[m], yp)
                    sqt = pa.tile([128, LQ], F32R, name=f"sq1_{m}", tag="sq", bufs=2)
                    nc.vector.tensor_mul(sqt, s_sb[m], s_sb[m])
                    mm(ssp, _r(onesK), _r(sqt), start=(m == 0), stop=(m == NT - 1))
                row = pa.tile([1, 512], F32R, name="row1", tag="row", bufs=2)
                nc.scalar.activation(row, ssp, AF.Sqrt, scale=1.0 / WIDTH, bias=eps1)
                with nc.allow_low_precision(reason="tf32 rstd"):
                    nc.vector.reciprocal(row, row)
                bcn = psa.tile([128, 512], F32, name="bcn", tag="vac", bufs=4)
                mm(bcn, _r(ones1), _r(row))
                for m in range(NT):
                    nc.vector.scalar_tensor_tensor(tin[m][:, 0:LQ], s_sb[m], g1[:, m:m + 1],
                                                   bcn, op0=OP.mult, op1=OP.mult)

        # ---------------- Stage C: SwiGLU FFN + RMSNorm2 ----------------
        # Two 1024-token blocks; the pure-rest block (cols LQ+REST/3..) has no
        # dependency on stage A/B, so it is emitted first.
        with tc.tile_pool(name="pcn", bufs=1) as pcn, \
             tc.tile_pool(name="wc", bufs=1) as wc, \
             tc.tile_pool(name="psc", bufs=1, space="PSUM") as psc:
            for blk in (1, 0):
                c0 = blk * TBLK
                F_t = [pcn.tile([128, TBLK], BF16, name=f"F_{blk}_{h}", tag=f"F{h}")
                       for h in range(NH)]
                # gate/up: weights streamed once per block, hidden-major
                for hm in range(NH):
                    wgb = wc.tile([128, NT, 128], BF16, name=f"wgb_{blk}_{hm}", tag="wgu", bufs=4)
                    nc.sync.dma_start(wgb, wg_r[:, :, hm * 128:(hm + 1) * 128])
                    wub = wc.tile([128, NT, 128], BF16, name=f"wub_{blk}_{hm}", tag="wgu", bufs=4)
                    nc.sync.dma_start(wub, wu_r[:, :, hm * 128:(hm + 1) * 128])
                    for j in range(NJ):
                        sl = slice(c0 + j * 512, c0 + (j + 1) * 512)
                        gp = psc.tile([128, 512], F32, name=f"gp_{blk}_{hm}_{j}", tag="gu", bufs=3)
                        for k in range(NT):
                            mm(gp, wgb[:, k, :], tin[k][:, sl], start=(k == 0), stop=(k == NT - 1))
                        up = psc.tile([128, 512], F32, name=f"up_{blk}_{hm}_{j}", tag="gu", bufs=3)
                        for k in range(NT):
                            mm(up, wub[:, k, :], tin[k][:, sl], start=(k == 0), stop=(k == NT - 1))
                        sg = pcn.tile([128, 512], F32R, name=f"sg_{blk}_{hm}_{j}", tag="sg", bufs=3)
                        nc.scalar.activation(sg, gp, AF.Silu)
                        nc.vector.tensor_mul(F_t[hm][:, j * 512:(j + 1) * 512], sg, up)
                # down + residual + RMSNorm2
                s2l = [pcn.tile([128, TBLK], F32R, name=f"s2_{blk}_{m}", tag=f"s2_{m}")
                       for m in range(NT)]
                ssp2 = [psc.tile([1, 512], F32, name=f"ssp2_{blk}_{j}", tag="st", bufs=2)
                        for j in range(NJ)]
                for m in range(NT):
                    wdb = []
                    for hh in range(2):
                        w = wc.tile([128, 16, 128], BF16, name=f"wdb_{blk}_{m}_{hh}", tag="wd", bufs=4)
                        nc.sync.dma_start(w, wd_r[:, hh * 16:(hh + 1) * 16, m * 128:(m + 1) * 128])
                        wdb.append(w)
                    for j in range(NJ):
                        sl = slice(c0 + j * 512, c0 + (j + 1) * 512)
                        dp = psc.tile([128, 512], F32, name=f"dp_{blk}_{m}_{j}", tag="d", bufs=3)
                        for hm in range(NH):
                            mm(dp, wdb[hm // 16][:, hm % 16, :], F_t[hm][:, j * 512:(j + 1) * 512],
                               start=(hm == 0), stop=(hm == NH - 1))
                        nc.vector.tensor_add(s2l[m][:, j * 512:(j + 1) * 512], tin[m][:, sl], dp)
                        sqt = pcn.tile([128, 512], F32R, name=f"sq2_{blk}_{m}_{j}", tag="sq2", bufs=3)
                        nc.vector.tensor_mul(sqt, s2l[m][:, j * 512:(j + 1) * 512],
                                             s2l[m][:, j * 512:(j + 1) * 512])
                        mm(ssp2[j], _r(onesK), _r(sqt), start=(m == 0), stop=(m == NT - 1))
                for j in range(NJ):
                    row2 = pcn.tile([1, 512], F32R, name=f"row2_{blk}_{j}", tag="row2", bufs=2)
                    nc.scalar.activation(row2, ssp2[j], AF.Sqrt, scale=1.0 / WIDTH, bias=eps1)
                    with nc.allow_low_precision(reason="tf32 rstd"):
                        nc.vector.reciprocal(row2, row2)
                    bc2 = psc.tile([128, 512], F32, name=f"bc2_{blk}_{j}", tag="gu", bufs=3)
                    mm(bc2, _r(ones1), _r(row2))
                    for m in range(NT):
                        ot = pcn.tile([128, 512], F32R, name=f"ot_{blk}_{j}_{m}", tag="ot", bufs=3)
                        nc.vector.scalar_tensor_tensor(ot, s2l[m][:, j * 512:(j + 1) * 512],
                                                       g2[:, m:m + 1], bc2,
                                                       op0=OP.mult, op1=OP.mult)
                        nc.sync.dma_start(out_r[m][:, c0 + j * 512:c0 + (j + 1) * 512], ot)


_INPUT_SPECS = [
    ("xw_t", [WIDTH, LWIN], BF16),
    ("xq_t", [WIDTH, LQ], BF16),
    ("xr_t", [WIDTH, REST], BF16),
    ("wq_t", [WIDTH, WIDTH], BF16),
    ("wk_t", [WIDTH, WIDTH], BF16),
    ("wv_t", [WIDTH, WIDTH], BF16),
    ("wo_t", [WIDTH, WIDTH], BF16),
    ("wg_t", [WIDTH, HID], BF16),
    ("wu_t", [WIDTH, HID], BF16),
    ("wd_t", [HID, WIDTH], BF16),
    ("cos_q", [128, LQ], F32),
    ("sin_q", [128, LQ], F32),
    ("cos_k", [128, LWIN], F32),
    ("sin_k", [128, LWIN], F32),
    ("g1", [128, NT], F32R),
    ("g2", [128, NT], F32R),
]


def build_program(reps=1):
    nc = bacc.Bacc("TRN2", target_bir_lowering=False, debug=False, num_devices=N_CORES)
    A = {name: nc.dram_tensor(name, shape, dt, kind="ExternalInput").ap()
         for name, shape, dt in _INPUT_SPECS}
    out_ap = nc.dram_tensor("out_t", [WIDTH, TOUT], F32R, kind="ExternalOutput").ap()
    with tile.TileContext(nc) as tc:
        for _ in range(reps):
            _emit(tc, A, out_ap)
    nc.compile()
    return nc


def make_in_maps(x, w_qkv, w_out, g_norm1, g_norm2, w_gate, w_up, w_down):
    f32 = np.float32
    bf = ml_dtypes.bfloat16
    x = np.asarray(x, f32)
    w_qkv = np.asarray(w_qkv, f32)
    # head-contiguous de-interleave: within head h, even dims first then odd:
    # new row h*64+j -> old h*64+2j ; new row h*64+32+j -> old h*64+2j+1
    perm = np.empty(WIDTH, np.int64)
    for h in range(HEADS):
        j = np.arange(32)
        perm[h * 64 + j] = h * 64 + 2 * j
        perm[h * 64 + 32 + j] = h * 64 + 2 * j + 1
    wq = w_qkv[0:WIDTH][perm]
    wk = w_qkv[WIDTH:2 * WIDTH][perm]
    wv = w_qkv[2 * WIDTH:3 * WIDTH]

    inv_freq = (1.0 / (ROPE_BASE ** (np.arange(0, HDIM, 2, dtype=np.float64) / HDIM)))

    def tab(pos):
        fr = np.outer(inv_freq, pos.astype(np.float64))  # [32, T]
        return (np.tile(np.cos(fr), (4, 1)).astype(f32),
                np.tile(np.sin(fr), (4, 1)).astype(f32))

    cos_k, sin_k = tab(np.arange(LWIN))
    common = {
        "wq_t": np.ascontiguousarray(wq.T.astype(bf)),
        "wk_t": np.ascontiguousarray(wk.T.astype(bf)),
        "wv_t": np.ascontiguousarray(wv.T.astype(bf)),
        "wo_t": np.ascontiguousarray(np.asarray(w_out, f32).T.astype(bf)),
        "wg_t": np.ascontiguousarray(np.asarray(w_gate, f32).T.astype(bf)),
        "wu_t": np.ascontiguousarray(np.asarray(w_up, f32).T.astype(bf)),
        "wd_t": np.ascontiguousarray(np.asarray(w_down, f32).T.astype(bf)),
        "cos_k": cos_k,
        "sin_k": sin_k,
        "g1": np.ascontiguousarray(np.asarray(g_norm1, f32).reshape(NT, 128).T),
        "g2": np.ascontiguousarray(np.asarray(g_norm2, f32).reshape(NT, 128).T),
    }
    in_maps = []
    for c in range(N_CORES):
        b, qh = c // 2, c % 2
        cos_q, sin_q = tab(np.arange(qh * LQ, (qh + 1) * LQ))
        m = dict(common)
        m["xw_t"] = np.ascontiguousarray(x[b, :LWIN].T.astype(bf))
        m["xq_t"] = np.ascontiguousarray(x[b, qh * LQ:(qh + 1) * LQ].T.astype(bf))
        m["xr_t"] = np.ascontiguousarray(x[b, LWIN + qh * REST:LWIN + (qh + 1) * REST].T.astype(bf))
        m["cos_q"] = cos_q
        m["sin_q"] = sin_q
        in_maps.append(m)
    return in_maps


def assemble_output(results):
    out = np.empty((4, 4096, WIDTH), np.float32)
    for c in range(N_CORES):
        b, qh = c // 2, c % 2
        o = results[c]["out_t"]
        out[b, qh * LQ:(qh + 1) * LQ] = o[:, :LQ].T
        out[b, LWIN + qh * REST:LWIN + (qh + 1) * REST] = o[:, LQ:].T
    return out


_CACHE = {}


def kernel(x, w_qkv, w_out, g_norm1, g_norm2, w_gate, w_up, w_down):
    if "nc" not in _CACHE:
        _CACHE["nc"] = build_program()
    nc = _CACHE["nc"]
    in_maps = make_in_maps(x, w_qkv, w_out, g_norm1, g_norm2, w_gate, w_up, w_down)
    res = run_bass_kernel_spmd(nc, in_maps, list(range(N_CORES))).results
    return assemble_output(res)


# revision 6
# speedup vs baseline: 1.3289x; 1.0915x over previous
"""Trainium2 Bass kernel for ContextualAttentionBlock.

Sharding: 8 cores, core c -> (batch b = c//2, query-half qh = c%2).
Each core computes, for its batch's 1024-token attention window:
  K/V projections for all 1024 tokens, Q for its 512 queries, RoPE,
  attention, out-proj, residual+RMSNorm1 -> h1 (512 tokens),
then SwiGLU FFN + residual + RMSNorm2 for 2048 tokens
  (512 attention-part tokens + 1536 "rest" tokens that skip attention).

All activations are feature-major ([feature, token]) so every matmul
contracts over the partition dim.  Matmul operands are bf16 (weights
cast on host); accumulation stays fp32.  The FFN streams gate/up/down
weights once per 1024-token block (hidden-major loop), and most of the
pure-"rest" block's gate/up tiles are interleaved into the attention
groups in program order so the PE stays dense through attention's
serial RoPE/softmax chains (which otherwise idle the PE and trip the
HAM clock throttle).  Softmax normalization is decoupled from the AV
PSUM accumulators (unnormalized copy out, scale in place later) so the
slow [1,N] vector reciprocal never gates PSUM bank reuse.  Residual
adds run in place (attention: into xq; FFN: into tin) to fit the
interleaved working set in SBUF.  One PSUM pool with 4 double-buffered
tags (proj/sc/vac/gu) is shared by all stages: 8 banks total.
No collectives; the host shards inputs and reassembles the output.
"""

import numpy as np
import ml_dtypes

import concourse.bass as bass
import concourse.tile as tile
from concourse import bacc, mybir
from concourse.bass_utils import run_bass_kernel_spmd

F32 = mybir.dt.float32
F32R = mybir.dt.float32r
BF16 = mybir.dt.bfloat16
AF = mybir.ActivationFunctionType
OP = mybir.AluOpType

WIDTH = 1024
NT = 8              # width tiles of 128
HEADS = 16
HDIM = 64
LWIN = 1024         # attention window
LQ = 512            # queries per core
HID = 4096
NH = 32             # hidden tiles of 128
REST = 1536         # rest tokens per core
TBLK = 1024         # ffn token block
NJ = 2              # 512-token slices per block
TOUT = LQ + REST
EPS = 1e-6
ROPE_BASE = 10000.0
N_CORES = 8
NHI = 5             # ffn hidden tiles interleaved per attention group


def _r(ap):
    return ap.bitcast(mybir.dt.float32r)


def _emit(tc, A, out_ap):
    nc = tc.nc
    mm = nc.tensor.matmul

    xw_r = A["xw_t"].rearrange("(a p) t -> a p t", p=128)
    xq_r = A["xq_t"].rearrange("(a p) t -> a p t", p=128)
    xr_r = A["xr_t"].rearrange("(a p) t -> a p t", p=128)
    wq_r = A["wq_t"].rearrange("(a p) m -> p a m", p=128)
    wk_r = A["wk_t"].rearrange("(a p) m -> p a m", p=128)
    wv_r = A["wv_t"].rearrange("(a p) m -> p a m", p=128)
    wo_r = A["wo_t"].rearrange("(a p) m -> p a m", p=128)
    wg_r = A["wg_t"].rearrange("(a p) m -> p a m", p=128)
    wu_r = A["wu_t"].rearrange("(a p) m -> p a m", p=128)
    wd_r = A["wd_t"].rearrange("(a p) m -> p a m", p=128)
    out_r = out_ap.rearrange("(a p) t -> a p t", p=128)

    with tc.tile_pool(name="pc", bufs=1) as pc, \
         tc.tile_pool(name="pf", bufs=1) as pf, \
         tc.tile_pool(name="ps", bufs=1, space="PSUM") as ps:
        # xq first: Q-proj is the first matmul work available.
        xq = [pc.tile([128, LQ], BF16, name=f"xq_{k}", tag=f"xq_{k}") for k in range(NT)]
        for k in range(NT):
            nc.sync.dma_start(xq[k], xq_r[k])
        g1 = pc.tile([128, NT], F32R, name="g1")
        nc.sync.dma_start(g1, A["g1"])
        g2 = pc.tile([128, NT], F32R, name="g2")
        nc.sync.dma_start(g2, A["g2"])
        onesF = pc.tile([128, 128], F32, name="onesF")
        nc.vector.memset(onesF, 1.0)
        onesK = pc.tile([128, 1], F32R, name="onesK")
        nc.vector.tensor_copy(onesK, onesF[:, 0:1])
        ones1 = pc.tile([1, 128], F32R, name="ones1")
        nc.vector.tensor_copy(ones1, onesF[0:1, :])
        eps1 = pc.tile([1, 1], F32, name="eps1")
        nc.vector.memset(eps1, EPS)
        ones64 = pc.tile([1, 64], F32R, name="ones64")
        nc.vector.tensor_copy(ones64, onesF[0:1, 0:64])
        ao = [pc.tile([128, LQ], BF16, name=f"ao_{i}", tag=f"ao{i}") for i in range(NT)]
        # FFN input, feature-major: cols [0:LQ] = h1 (written by stage B),
        # cols [LQ:] = raw rest tokens (DMA'd now).
        tin = [pc.tile([128, TOUT], BF16, name=f"tin_{m}", tag=f"tin_{m}")
               for m in range(NT)]
        for m in range(NT):
            nc.sync.dma_start(tin[m][:, LQ:], xr_r[m])

        def ffn_gate_up(blk, hm):
            c0 = blk * TBLK
            wgb = pf.tile([128, NT, 128], BF16, name=f"wgb_{blk}_{hm}", tag="wgu", bufs=4)
            nc.sync.dma_start(wgb, wg_r[:, :, hm * 128:(hm + 1) * 128])
            wub = pf.tile([128, NT, 128], BF16, name=f"wub_{blk}_{hm}", tag="wgu", bufs=4)
            nc.sync.dma_start(wub, wu_r[:, :, hm * 128:(hm + 1) * 128])
            F_t = pf.tile([128, TBLK], BF16, name=f"F_{blk}_{hm}", tag=f"F{hm}")
            for j in range(NJ):
                sl = slice(c0 + j * 512, c0 + (j + 1) * 512)
                gp = ps.tile([128, 512], F32, name=f"gp_{blk}_{hm}_{j}", tag="gu", bufs=2)
                for k in range(NT):
                    mm(gp, wgb[:, k, :], tin[k][:, sl], start=(k == 0), stop=(k == NT - 1))
                up = ps.tile([128, 512], F32, name=f"up_{blk}_{hm}_{j}", tag="gu", bufs=2)
                for k in range(NT):
                    mm(up, wub[:, k, :], tin[k][:, sl], start=(k == 0), stop=(k == NT - 1))
                sg = pf.tile([128, 512], F32R, name=f"sg_{blk}_{hm}_{j}", tag="sg", bufs=3)
                nc.scalar.activation(sg, gp, AF.Silu)
                nc.vector.tensor_mul(F_t[:, j * 512:(j + 1) * 512], sg, up)
            return F_t

        # ============ Stage A: attention + interleaved FFN gate/up ==========
        F1 = [None] * NH  # block-1 (pure-rest) gate/up outputs
        with tc.tile_pool(name="pa", bufs=1) as pa, \
             tc.tile_pool(name="wa", bufs=1) as wa:
            cq = pa.tile([128, LQ], BF16, name="cq")
            nc.sync.dma_start(cq, A["cos_q"])
            sq = pa.tile([128, LQ], BF16, name="sq")
            nc.sync.dma_start(sq, A["sin_q"])
            ck = pa.tile([128, LWIN], BF16, name="ck")
            nc.sync.dma_start(ck, A["cos_k"])
            sk = pa.tile([128, LWIN], BF16, name="sk")
            nc.sync.dma_start(sk, A["sin_k"])
            xw = [pa.tile([128, LWIN], BF16, name=f"xw_{k}", tag=f"xw{k}") for k in range(NT)]
            for k in range(NT):
                nc.sync.dma_start(xw[k], xw_r[k])

            def rope_from_psum(ps_in, dest, cos, sin, scratch):
                # per 64-row head block: rows b..b+32 = even dims E,
                # rows b+32..b+64 = odd dims O (head-contiguous perm).
                # dest[b:b+32]    = E*cos - O*sin
                # dest[b+32:b+64] = O*cos + E*sin
                # cos/sin rows repeat every 32 (freq = row % 32), so the
                # SBUF operand can always share the output's base partition.
                for b in (0, 64):
                    e_ps, o_ps = ps_in[b:b + 32, :], ps_in[b + 32:b + 64, :]
                    nc.vector.tensor_mul(scratch[b + 32:b + 64, :], e_ps,
                                         sin[b + 32:b + 64, :])  # E*sin
                    nc.vector.tensor_mul(scratch[b:b + 32, :], o_ps,
                                         sin[b:b + 32, :])       # O*sin
                    nc.vector.tensor_mul(dest[b:b + 32, :], e_ps, cos[b:b + 32, :])
                    nc.vector.tensor_sub(dest[b:b + 32, :], dest[b:b + 32, :],
                                         scratch[b:b + 32, :])
                    nc.vector.tensor_mul(dest[b + 32:b + 64, :], o_ps,
                                         cos[b + 32:b + 64, :])
                    nc.vector.tensor_add(dest[b + 32:b + 64, :],
                                         dest[b + 32:b + 64, :],
                                         scratch[b + 32:b + 64, :])

            for g in range(4):
                # --- Q projection + RoPE (tiles 2g, 2g+1; heads 4g..4g+3) ---
                q2 = []
                for side, m in ((0, 2 * g), (1, 2 * g + 1)):
                    wb = wa.tile([128, NT, 128], BF16, name=f"wqb_{g}_{side}", tag="wqk", bufs=2)
                    nc.sync.dma_start(wb, wq_r[:, :, m * 128:(m + 1) * 128])
                    psq = ps.tile([128, LQ], F32, name=f"qps_{g}_{side}", tag="proj", bufs=2)
                    for k in range(NT):
                        mm(psq, wb[:, k, :], xq[k], start=(k == 0), stop=(k == NT - 1))
                    qt = pa.tile([128, LQ], BF16, name=f"q_{g}_{side}",
                                 tag=("q0" if side == 0 else "q1"), bufs=1)
                    qs = pa.tile([128, LQ], F32R, name=f"qs_{g}_{side}", tag="rs", bufs=2)
                    rope_from_psum(psq, qt, cq, sq, qs)
                    q2.append(qt)

                # --- K projection + RoPE ---
                k2 = []
                for side, m in ((0, 2 * g), (1, 2 * g + 1)):
                    wb = wa.tile([128, NT, 128], BF16, name=f"wkb_{g}_{side}", tag="wqk", bufs=2)
                    nc.sync.dma_start(wb, wk_r[:, :, m * 128:(m + 1) * 128])
                    kt_sb = pa.tile([128, LWIN], BF16, name=f"k_{g}_{side}",
                                    tag=("k0" if side == 0 else "k1"), bufs=1)
                    for ch in range(2):
                        psk = ps.tile([128, 512], F32, name=f"kps_{g}_{side}_{ch}", tag="proj", bufs=2)
                        for k in range(NT):
                            mm(psk, wb[:, k, :], xw[k][:, ch * 512:(ch + 1) * 512],
                               start=(k == 0), stop=(k == NT - 1))
                        ks = pa.tile([128, 512], F32R, name=f"ks_{g}_{side}_{ch}", tag="rs", bufs=2)
                        rope_from_psum(psk, kt_sb[:, ch * 512:(ch + 1) * 512],
                                       ck[:, ch * 512:(ch + 1) * 512],
                                       sk[:, ch * 512:(ch + 1) * 512], ks)
                    k2.append(kt_sb)

                # --- V projection (token-major, 65-col per head, ones col) ---
                wvb = wa.tile([128, NT, 256], BF16, name=f"wvb_{g}", tag="wv", bufs=1)
                nc.sync.dma_start(wvb, wv_r[:, :, g * 256:(g + 1) * 256])
                vg = [pa.tile([128, 4 * 65], BF16, name=f"v_{g}_{kt}", tag=f"v{kt}", bufs=1)
                      for kt in range(NT)]
                for kt in range(NT):
                    psv = ps.tile([128, 256], F32, name=f"vps_{g}_{kt}", tag="proj", bufs=2)
                    for k in range(NT):
                        mm(psv, xw[k][:, kt * 128:(kt + 1) * 128], wvb[:, k, :],
                           start=(k == 0), stop=(k == NT - 1))
                    v3 = vg[kt].rearrange("p (h c) -> p h c", c=65)
                    nc.vector.tensor_copy(v3[:, :, 64:65],
                                          onesF[:, 0:1].unsqueeze(1).broadcast_to([128, 4, 1]))
                    nc.vector.tensor_copy(v3[:, :, 0:64], psv.rearrange("p (h c) -> p h c", c=64))

                # --- attention per head pair ---
                for p2 in range(2):
                    vac = [ps.tile([65, 512], F32, name=f"vac_{g}_{p2}_{jj}", tag="vac", bufs=2)
                           for jj in range(2)]
                    for kt in range(NT):
                        for jj in range(2):
                            j = 2 * p2 + jj
                            sc = ps.tile([128, 512], F32, name=f"sc_{g}_{p2}_{kt}_{jj}",
                                         tag="sc", bufs=2)
                            mm(sc, k2[p2][64 * jj:64 * (jj + 1), kt * 128:(kt + 1) * 128],
                               q2[p2][64 * jj:64 * (jj + 1), :],
                               start=True, stop=True, tile_position=(64 * jj, 0))
                            at = pa.tile([128, 512], BF16, name=f"at_{g}_{p2}_{kt}_{jj}",
                                         tag="at", bufs=3)
                            nc.scalar.activation(at, sc, AF.Exp, scale=0.125)
                            mm(vac[jj], vg[kt][:, j * 65:(j + 1) * 65], at,
                               start=(kt == 0), stop=(kt == NT - 1))
                    # drain vac promptly: unnormalized out + denominator row;
                    # normalize in place once the reciprocal lands, without
                    # holding the vac banks.
                    i = 2 * g + p2
                    nc.scalar.copy(ao[i][0:64, :], vac[0][0:64, :])
                    nc.scalar.copy(ao[i][64:128, :], vac[1][0:64, :])
                    rr = pa.tile([1, 1024], F32R, name=f"rr_{g}_{p2}", tag="rr", bufs=2)
                    nc.scalar.copy(rr[0:1, 0:512], vac[0][64:65, :])
                    nc.scalar.copy(rr[0:1, 512:1024], vac[1][64:65, :])
                    with nc.allow_low_precision(reason="tf32 softmax denom"):
                        nc.vector.reciprocal(rr, rr)
                    bc0 = ps.tile([64, 512], F32, name=f"bca_{g}_{p2}_0", tag="sc", bufs=2)
                    mm(bc0, _r(ones64), _r(rr[0:1, 0:512]))
                    bc1 = ps.tile([64, 512], F32, name=f"bca_{g}_{p2}_1", tag="vac", bufs=2)
                    mm(bc1, _r(ones64), _r(rr[0:1, 512:1024]))
                    bcs = pa.tile([128, 512], F32R, name=f"bcs_{g}_{p2}", tag="bcs", bufs=1)
                    nc.scalar.copy(bcs[0:64, :], bc0)
                    nc.scalar.copy(bcs[64:128, :], bc1)
                    nc.vector.tensor_mul(ao[i][0:64, :], ao[i][0:64, :], bcs[0:64, :])
                    nc.vector.tensor_mul(ao[i][64:128, :], ao[i][64:128, :], bcs[64:128, :])

                # --- interleaved FFN gate/up for the pure-rest block ---
                for hm in range(g * NHI, (g + 1) * NHI):
                    F1[hm] = ffn_gate_up(1, hm)

            # ---------- Stage B: out-proj + RMSNorm1 -> h1 (into tin) -------
            ssp = ps.tile([1, 512], F32, name="ssp", tag="sc", bufs=2)
            for m in range(NT):
                wb = wa.tile([128, NT, 128], BF16, name=f"wob_{m}", tag="wqk", bufs=2)
                nc.sync.dma_start(wb, wo_r[:, :, m * 128:(m + 1) * 128])
                yp = ps.tile([128, LQ], F32, name=f"yps_{m}", tag="proj", bufs=2)
                for k in range(NT):
                    mm(yp, wb[:, k, :], ao[k], start=(k == 0), stop=(k == NT - 1))
                nc.vector.tensor_add(xq[m], xq[m], yp)  # residual, in place
                sqt = pa.tile([128, LQ], F32R, name=f"sq1_{m}", tag="sqA", bufs=2)
                nc.vector.tensor_mul(sqt, xq[m], xq[m])
                mm(ssp, _r(onesK), _r(sqt), start=(m == 0), stop=(m == NT - 1))
            row = pa.tile([1, 512], F32R, name="row1", tag="row", bufs=1)
            nc.scalar.activation(row, ssp, AF.Sqrt, scale=1.0 / WIDTH, bias=eps1)
            with nc.allow_low_precision(reason="tf32 rstd"):
                nc.vector.reciprocal(row, row)
            bcn = ps.tile([128, 512], F32, name="bcn", tag="vac", bufs=2)
            mm(bcn, _r(ones1), _r(row))
            for m in range(NT):
                nc.vector.scalar_tensor_tensor(tin[m][:, 0:LQ], xq[m], g1[:, m:m + 1],
                                               bcn, op0=OP.mult, op1=OP.mult)

        # ================= Stage C: remaining FFN =======================
        def ffn_down(blk, F_ts, wc, pd):
            c0 = blk * TBLK
            ssp2 = [ps.tile([1, 512], F32, name=f"ssp2_{blk}_{j}", tag="sc", bufs=2)
                    for j in range(NJ)]
            for m in range(NT):
                wdb = []
                for hh in range(2):
                    w = wc.tile([128, 16, 128], BF16, name=f"wdb_{blk}_{m}_{hh}", tag="wd", bufs=4)
                    nc.sync.dma_start(w, wd_r[:, hh * 16:(hh + 1) * 16, m * 128:(m + 1) * 128])
                    wdb.append(w)
                for j in range(NJ):
                    sl = slice(c0 + j * 512, c0 + (j + 1) * 512)
                    dp = ps.tile([128, 512], F32, name=f"dp_{blk}_{m}_{j}", tag="proj", bufs=2)
                    for hm in range(NH):
                        mm(dp, wdb[hm // 16][:, hm % 16, :], F_ts[hm][:, j * 512:(j + 1) * 512],
                           start=(hm == 0), stop=(hm == NH - 1))
                    nc.vector.tensor_add(tin[m][:, sl], tin[m][:, sl], dp)  # in place
                    sqt = pd.tile([128, 512], F32R, name=f"sq2_{blk}_{m}_{j}", tag="sq2", bufs=3)
                    nc.vector.tensor_mul(sqt, tin[m][:, sl], tin[m][:, sl])
                    mm(ssp2[j], _r(onesK), _r(sqt), start=(m == 0), stop=(m == NT - 1))
            for j in range(NJ):
                row2 = pd.tile([1, 512], F32R, name=f"row2_{blk}_{j}", tag="row2", bufs=2)
                nc.scalar.activation(row2, ssp2[j], AF.Sqrt, scale=1.0 / WIDTH, bias=eps1)
                with nc.allow_low_precision(reason="tf32 rstd"):
                    nc.vector.reciprocal(row2, row2)
                bc2 = ps.tile([128, 512], F32, name=f"bc2_{blk}_{j}", tag="vac", bufs=2)
                mm(bc2, _r(ones1), _r(row2))
                for m in range(NT):
                    ot = pd.tile([128, 512], F32R, name=f"ot_{blk}_{j}_{m}", tag="ot", bufs=3)
                    nc.vector.scalar_tensor_tensor(ot, tin[m][:, c0 + j * 512:c0 + (j + 1) * 512],
                                                   g2[:, m:m + 1], bc2,
                                                   op0=OP.mult, op1=OP.mult)
                    nc.sync.dma_start(out_r[m][:, c0 + j * 512:c0 + (j + 1) * 512], ot)

        with tc.tile_pool(name="pd", bufs=1) as pd, \
             tc.tile_pool(name="wc", bufs=1) as wc:
            for hm in range(4 * NHI, NH):
                F1[hm] = ffn_gate_up(1, hm)
            ffn_down(1, F1, wc, pd)
            F0 = [ffn_gate_up(0, hm) for hm in range(NH)]
            ffn_down(0, F0, wc, pd)


_INPUT_SPECS = [
    ("xw_t", [WIDTH, LWIN], BF16),
    ("xq_t", [WIDTH, LQ], BF16),
    ("xr_t", [WIDTH, REST], BF16),
    ("wq_t", [WIDTH, WIDTH], BF16),
    ("wk_t", [WIDTH, WIDTH], BF16),
    ("wv_t", [WIDTH, WIDTH], BF16),
    ("wo_t", [WIDTH, WIDTH], BF16),
    ("wg_t", [WIDTH, HID], BF16),
    ("wu_t", [WIDTH, HID], BF16),
    ("wd_t", [HID, WIDTH], BF16),
    ("cos_q", [128, LQ], BF16),
    ("sin_q", [128, LQ], BF16),
    ("cos_k", [128, LWIN], BF16),
    ("sin_k", [128, LWIN], BF16),
    ("g1", [128, NT], F32R),
    ("g2", [128, NT], F32R),
]


def build_program(reps=1):
    nc = bacc.Bacc("TRN2", target_bir_lowering=False, debug=False, num_devices=N_CORES)
    A = {name: nc.dram_tensor(name, shape, dt, kind="ExternalInput").ap()
         for name, shape, dt in _INPUT_SPECS}
    out_ap = nc.dram_tensor("out_t", [WIDTH, TOUT], F32R, kind="ExternalOutput").ap()
    with tile.TileContext(nc) as tc:
        for _ in range(reps):
            _emit(tc, A, out_ap)
    nc.compile()
    return nc


def make_in_maps(x, w_qkv, w_out, g_norm1, g_norm2, w_gate, w_up, w_down):
    f32 = np.float32
    bf = ml_dtypes.bfloat16
    x = np.asarray(x, f32)
    w_qkv = np.asarray(w_qkv, f32)
    # head-contiguous de-interleave: within head h, even dims first then odd:
    # new row h*64+j -> old h*64+2j ; new row h*64+32+j -> old h*64+2j+1
    perm = np.empty(WIDTH, np.int64)
    for h in range(HEADS):
        j = np.arange(32)
        perm[h * 64 + j] = h * 64 + 2 * j
        perm[h * 64 + 32 + j] = h * 64 + 2 * j + 1
    wq = w_qkv[0:WIDTH][perm]
    wk = w_qkv[WIDTH:2 * WIDTH][perm]
    wv = w_qkv[2 * WIDTH:3 * WIDTH]

    inv_freq = (1.0 / (ROPE_BASE ** (np.arange(0, HDIM, 2, dtype=np.float64) / HDIM)))

    def tab(pos):
        fr = np.outer(inv_freq, pos.astype(np.float64))  # [32, T]
        return (np.tile(np.cos(fr), (4, 1)).astype(bf),
                np.tile(np.sin(fr), (4, 1)).astype(bf))

    cos_k, sin_k = tab(np.arange(LWIN))
    common = {
        "wq_t": np.ascontiguousarray(wq.T.astype(bf)),
        "wk_t": np.ascontiguousarray(wk.T.astype(bf)),
        "wv_t": np.ascontiguousarray(wv.T.astype(bf)),
        "wo_t": np.ascontiguousarray(np.asarray(w_out, f32).T.astype(bf)),
        "wg_t": np.ascontiguousarray(np.asarray(w_gate, f32).T.astype(bf)),
        "wu_t": np.ascontiguousarray(np.asarray(w_up, f32).T.astype(bf)),
        "wd_t": np.ascontiguousarray(np.asarray(w_down, f32).T.astype(bf)),
        "cos_k": cos_k,
        "sin_k": sin_k,
        "g1": np.ascontiguousarray(np.asarray(g_norm1, f32).reshape(NT, 128).T),
        "g2": np.ascontiguousarray(np.asarray(g_norm2, f32).reshape(NT, 128).T),
    }
    in_maps = []
    for c in range(N_CORES):
        b, qh = c // 2, c % 2
        cos_q, sin_q = tab(np.arange(qh * LQ, (qh + 1) * LQ))
        m = dict(common)
        m["xw_t"] = np.ascontiguousarray(x[b, :LWIN].T.astype(bf))
        m["xq_t"] = np.ascontiguousarray(x[b, qh * LQ:(qh + 1) * LQ].T.astype(bf))
        m["xr_t"] = np.ascontiguousarray(x[b, LWIN + qh * REST:LWIN + (qh + 1) * REST].T.astype(bf))
        m["cos_q"] = cos_q
        m["sin_q"] = sin_q
        in_maps.append(m)
    return in_maps


def assemble_output(results):
    out = np.empty((4, 4096, WIDTH), np.float32)
    for c in range(N_CORES):
        b, qh = c // 2, c % 2
        o = results[c]["out_t"]
        out[b, qh * LQ:(qh + 1) * LQ] = o[:, :LQ].T
        out[b, LWIN + qh * REST:LWIN + (qh + 1) * REST] = o[:, LQ:].T
    return out


_CACHE = {}


def kernel(x, w_qkv, w_out, g_norm1, g_norm2, w_gate, w_up, w_down):
    if "nc" not in _CACHE:
        _CACHE["nc"] = build_program()
    nc = _CACHE["nc"]
    in_maps = make_in_maps(x, w_qkv, w_out, g_norm1, g_norm2, w_gate, w_up, w_down)
    res = run_bass_kernel_spmd(nc, in_maps, list(range(N_CORES))).results
    return assemble_output(res)
